# revision 10
# baseline (speedup 1.0000x reference)
"""Multi-head self-attention (B=2, T=2048, D=1024, H=16) on 8 TRN2 NeuronCores.

Sharding: core c -> (b = c // 4, head-group hg = c % 4); each core computes the
full causal attention + partial output projection for its 4 heads of one batch
element.  The host pre-transposes x, pre-slices Wqkv columns / Wout rows per
head group, and sums the 4 partial projections per batch element (+ bout).

Device-side dataflow (per core), all matmuls bf16:
  A) qkT[c,t] = W[:,c].T @ xT  (c-major, chains of 8 contraction matmuls per
     (cb, it) psum tile; cb order q0,k0,q1,k1 so pair-0 attention can start
     as early as possible)
     V[t,c] = xT[:,t].T @ Wv  (+ones columns for row sums)
  B) it-major attention: S^T[j,i] = kT.T @ qT per (it, jb, pair) with the two
     heads of a pair row-packed (lhsT partitions 0:64 / 64:128); diagonal
     blocks only compute columns >= 128q (exact causal).
     P^T = exp(S^T/8) on ScalarE straight out of PSUM (both heads per call).
     ctx^T (+replicated sums rows) = [V|1].T @ P^T accumulated per pair.
     finish: DVE reciprocal on the 64-replicated sums rows straight from
     PSUM, one SBUF->SBUF partition-shift DMA, normalize on DVE.  ScalarE
     does nothing but exp in phase B.
  C) out[t,e] = ctx^T.T @ Wout_shard, interleaved per-it into phase B
     (shares the ps_s PSUM rotation), bf16 partial result back to host.
"""

import math
from contextlib import ExitStack

import numpy as np
import ml_dtypes

import concourse.bass as bass
import concourse.bacc as bacc_mod
import concourse.mybir as mybir
import concourse.tile as tile

FP32 = mybir.dt.float32
BF16 = mybir.dt.bfloat16
AF = mybir.ActivationFunctionType
ALU = mybir.AluOpType

B, T, D, H = 2, 2048, 1024, 16
Dh = D // H          # 64
NCORES = 8
HPC = 4              # heads per core
NPAIR = HPC // 2     # head pairs (2 heads share a 128-partition block)
IT = T // 512        # 4 query tiles of 512
JB = T // 128        # 16 key blocks of 128
KO = D // 128        # 8 contraction blocks for the projections
SCALE = 1.0 / math.sqrt(Dh)


def build_program(compile=True):
    nc = bacc_mod.Bacc()

    xT = nc.declare_dram_parameter("xT", [D, T], BF16, isOutput=False)
    wqk = nc.declare_dram_parameter("wqk", [128, KO, 2 * HPC * Dh], BF16,
                                    isOutput=False)
    wv = nc.declare_dram_parameter("wv", [128, KO, HPC * Dh], BF16,
                                   isOutput=False)
    wout = nc.declare_dram_parameter("wout", [128, 2, D], BF16, isOutput=False)
    # consts: [tri 128 | ones-col 64 | zeros 384]
    consts = nc.declare_dram_parameter("consts", [128, 576], BF16, isOutput=False)
    out = nc.declare_dram_parameter("out", [T, D], BF16, isOutput=True)

    xT_r = xT.rearrange("(o p) t -> p o t", p=128)

    with ExitStack() as ctx:
        tc = ctx.enter_context(tile.TileContext(nc))
        persist = ctx.enter_context(tc.tile_pool(name="persist", bufs=1))

        # ---------------- persistent tiles ----------------
        qkT = {}
        for nm in ("qT0", "kT0", "qT1", "kT1"):
            qkT[nm] = persist.tile([128, T], BF16, name=nm, tag=nm)
        V_aug = persist.tile([128, JB, HPC, 128], BF16, name="V_aug", tag="V_aug")
        merged = [
            persist.tile([128, IT, 512], BF16, name=f"merged{p}", tag=f"merged{p}")
            for p in range(NPAIR)
        ]
        wout_sb = persist.tile([128, 2, D], BF16, name="wout_sb", tag="wout_sb")
        consts_sb = persist.tile([128, 576], BF16, name="consts_sb",
                                 tag="consts_sb")
        tri = consts_sb[:, 0:128]

        diag_pT = {
            (q, pr): persist.tile([128, 2, 512], BF16, name=f"pTd{q}_{pr}",
                                  tag=f"pTd{q}_{pr}")
            for q in range(4) for pr in range(NPAIR)
        }

        def load_consts():
            # ones columns 64..127 of V_aug weights: the AV matmul then emits
            # the softmax denominators replicated on PSUM rows 64..127
            nc.vector.tensor_copy(
                V_aug[:, :, :, 64:],
                consts_sb[:, None, None, 128:192].to_broadcast(
                    (128, JB, HPC, 64)),
            )
            # pre-zero the fully-masked column prefix [0, 128q) of diagonal
            # P^T pair-tiles (exp only ever writes columns >= 128q; the
            # triangle multiply covers the square)
            for (q, pr), t_ in diag_pT.items():
                if q > 0:
                    for hl in range(2):
                        nc.vector.tensor_copy(
                            t_[:, hl, : 128 * q],
                            consts_sb[:, 192: 192 + 128 * q],
                        )

        # ---------------- phase A: QKV projections ----------------
        with (
            tc.tile_pool(name="phA", bufs=1) as pa,
            tc.tile_pool(name="psA", bufs=1, space="PSUM") as psa,
        ):
            xT_sb = pa.tile([128, KO, T], BF16, name="xT_sb", tag="xT_sb", bufs=1)
            wqk_sb = pa.tile([128, KO, 2 * HPC * Dh], BF16, name="wqk_sb",
                             tag="wqk_sb", bufs=1)
            wv_sb = pa.tile([128, KO, HPC * Dh], BF16, name="wv_sb", tag="wv_sb",
                            bufs=1)
            # interleave weight/x DMAs per o-block, it0 chunks first, so the
            # first matmul chain can start within ~2us
            for o in range(KO):
                nc.sync.dma_start(wqk_sb[:, o], wqk[:, o])
                nc.sync.dma_start(xT_sb[:, o, 0:512], xT_r[:, o, 0:512])
            nc.sync.dma_start(consts_sb[:], consts[:])
            load_consts()
            for it in range(1, IT):
                isl = slice(512 * it, 512 * (it + 1))
                for o in range(KO):
                    nc.sync.dma_start(xT_sb[:, o, isl], xT_r[:, o, isl])
            nc.sync.dma_start(wv_sb[:], wv[:])
            nc.sync.dma_start(wout_sb[:], wout[:])

            # qT/kT: [c, t] c-major; per (cb, it): one 8-matmul contraction
            # chain into one psum bank (LDWEIGHTS hides inside the chain)
            dests = [qkT["qT0"], qkT["kT0"], qkT["qT1"], qkT["kT1"]]
            cbs = [0, 2, 1, 3]  # wqk column blocks: q0, k0, q1, k1
            for di, cb in enumerate(cbs):
                for it in range(IT):
                    isl = slice(512 * it, 512 * (it + 1))
                    ps = psa.tile([128, 512], FP32, name="ps_a", tag="ps_a",
                                  bufs=3)
                    for o in range(KO):
                        nc.tensor.matmul(
                            ps[:],
                            lhsT=wqk_sb[:, o, 128 * cb: 128 * (cb + 1)],
                            rhs=xT_sb[:, o, isl],
                            start=(o == 0), stop=(o == KO - 1),
                        )
                    eng = nc.scalar if it % 2 == 0 else nc.vector
                    if eng is nc.scalar:
                        nc.scalar.copy(dests[di][:, isl], ps[:])
                    else:
                        nc.vector.tensor_copy(dests[di][:, isl], ps[:])

            # V natural [t, c] -> V_aug[:, tb, h, 0:64]
            for tb in range(JB):
                psv = psa.tile([128, HPC * Dh], FP32, name="ps_v", tag="ps_v",
                               bufs=2)
                for o in range(KO):
                    nc.tensor.matmul(
                        psv[:],
                        lhsT=xT_sb[:, o, 128 * tb: 128 * (tb + 1)],
                        rhs=wv_sb[:, o],
                        start=(o == 0), stop=(o == KO - 1),
                    )
                nc.vector.tensor_copy(
                    V_aug[:, tb, :, 0:64],
                    psv[:].rearrange("p (h d) -> p h d", h=HPC),
                )

        # ---------------- phase B: attention (+ phase C interleaved) --------
        with (
            tc.tile_pool(name="phB", bufs=2) as pb,
            tc.tile_pool(name="psB", bufs=1, space="PSUM") as psb,
        ):
            merged_flat = [m.rearrange("p a b -> p (a b)") for m in merged]

            def finish_pair(it, pair, psum_ctx):
                """Drain one pair's ctx^T and normalize, ScalarE-free.

                psum_ctx rows 0:64   = ctx^T (hl 0 = head even, 1 = odd)
                          rows 64:128 = sums replicated x64 per hl.
                """
                # full-partition reciprocal: rows 64:128 become 1/sums, rows
                # 0:64 are garbage (1/ctx) and never read
                recs = pb.tile([128, 2, 512], FP32, name="recs", tag="recs",
                               bufs=2)
                nc.vector.reciprocal_approx_fast(recs[:], psum_ctx[:])
                # unnormalized ctx out of PSUM (frees the accumulator fast)
                ctxU = pb.tile([64, 2, 512], BF16, name="ctxU", tag="ctxU",
                               bufs=2)
                nc.vector.tensor_copy(ctxU[:], psum_ctx[0:64, :, :])
                # shift recips to partitions 0:64 where ctx lives
                recsLo = pb.tile([64, 2, 512], FP32, name="recsLo",
                                 tag="recsLo", bufs=2)
                nc.sync.dma_start(recsLo[:], recs[64:128, :, :])
                # normalize + pack [h0 | h1] into merged partitions
                nc.vector.tensor_tensor(
                    out=merged[pair][0:64, it], in0=ctxU[:, 0, :],
                    in1=recsLo[:, 0, :], op=ALU.mult,
                )
                tmp = pb.tile([64, 512], BF16, name="odd_tmp", tag="odd_tmp",
                              bufs=2)
                nc.vector.tensor_tensor(
                    out=tmp[:], in0=ctxU[:, 1, :], in1=recsLo[:, 1, :],
                    op=ALU.mult,
                )
                nc.sync.dma_start(merged[pair][64:128, it], tmp[:])

            def phase_c_chunk_tb(it, k):
                """Output projection for t-block 4it+k (shares the ps_s PSUM
                rotation; emitted spread through the NEXT it's jb loop so it
                fills exp-wait gaps instead of serializing)."""
                tb = 4 * it + k
                tsl = slice(128 * tb, 128 * (tb + 1))
                pso = psb.tile([128, 2, 512], FP32, name="ps_o",
                               tag="ps_s", bufs=2)
                for pair in range(NPAIR):
                    for et in range(2):
                        nc.tensor.matmul(
                            pso[:, et, :],
                            lhsT=merged_flat[pair][:, tsl],
                            rhs=wout_sb[:, pair, 512 * et: 512 * (et + 1)],
                            start=(pair == 0), stop=(pair == NPAIR - 1),
                        )
                osb = pb.tile([128, 2, 512], BF16, name="osb", tag="osb",
                              bufs=3)
                nc.vector.tensor_copy(osb[:], pso[:])
                nc.sync.dma_start(
                    out[tsl, :], osb.rearrange("p a b -> p (a b)"))

            for it in range(IT):
                isl = slice(512 * it, 512 * (it + 1))
                njb = 4 * it + 4  # causal: j blocks 0 .. 4it+3
                ctxs = [
                    psb.tile([128, 2, 512], FP32, name="psum_ctx",
                             tag=f"psum_ctx{pair}", bufs=1)
                    for pair in range(NPAIR)
                ]
                # diagonal j-blocks first: the q=0 block opens the ctx
                # accumulation full-width, later diag blocks accumulate only
                # their causal column range, and the last full block closes
                # the group full-width (it=0 has no full blocks -> no trim)
                jb_order = list(range(4 * it, njb)) + list(range(4 * it))
                trim_av = it > 0
                for ji, jb in enumerate(jb_order):
                    if it > 0 and ji >= 2 and ji - 2 < 4:
                        phase_c_chunk_tb(it - 1, ji - 2)
                    jsl = slice(128 * jb, 128 * (jb + 1))
                    q = jb - 4 * it
                    c0 = 128 * q if q > 0 else 0  # exact-causal column start
                    av0 = c0 if trim_av else 0
                    for pair in range(NPAIR):
                        kT_t = qkT[f"kT{pair}"]
                        qT_t = qkT[f"qT{pair}"]
                        psum_ctx = ctxs[pair]
                        ps2 = psb.tile([128, 2, 512], FP32, name="ps_s",
                                       tag="ps_s", bufs=2)
                        # two heads row-packed (lhsT partitions 0:64 / 64:128)
                        for hl in range(2):
                            rows = slice(64 * hl, 64 * (hl + 1))
                            nc.tensor.matmul(
                                ps2[:, hl, c0:],
                                lhsT=kT_t[rows, jsl],
                                rhs=qT_t[rows, isl][:, c0:],
                                start=True, stop=True,
                            )
                        if q < 0:  # fully sub-diagonal block: plain exp
                            pT = pb.tile([128, 2, 512], BF16, name="pT",
                                         tag="pT_full", bufs=3)
                            nc.scalar.activation(pT[:], ps2[:], AF.Exp,
                                                 scale=SCALE)
                        else:      # diagonal-class block
                            pT = diag_pT[(q, pair)]
                            nc.scalar.activation(
                                pT[:, :, c0:], ps2[:, :, c0:],
                                AF.Exp, scale=SCALE,
                            )
                            for hl in range(2):
                                nc.vector.tensor_tensor(
                                    out=pT[:, hl, 128 * q: 128 * (q + 1)],
                                    in0=pT[:, hl, 128 * q: 128 * (q + 1)],
                                    in1=tri[:],
                                    op=ALU.mult,
                                )
                        for hl in range(2):
                            h = 2 * pair + hl
                            nc.tensor.matmul(
                                psum_ctx[:, hl, av0:],
                                lhsT=V_aug[:, jb, h, :],
                                rhs=pT[:, hl, av0:],
                                start=(ji == 0), stop=(ji == njb - 1),
                            )
                for pair in range(NPAIR):
                    finish_pair(it, pair, ctxs[pair])
            for k in range(4):
                phase_c_chunk_tb(IT - 1, k)

    if compile:
        nc.compile()
    return nc


_PROGRAM = None


def _get_program():
    global _PROGRAM
    if _PROGRAM is None:
        _PROGRAM = build_program()
    return _PROGRAM


def _consts():
    c = np.zeros((128, 576), ml_dtypes.bfloat16)
    dj = np.arange(128)[:, None]
    di = np.arange(128)[None, :]
    c[:, 0:128] = (dj <= di).astype(ml_dtypes.bfloat16)   # causal triangle
    c[:, 128:192] = 1.0                      # 64 ones columns
    return c


def make_in_maps(x, Wqkv, Wout):
    in_maps = []
    for core in range(NCORES):
        b, hg = core // (NCORES // B), core % (NCORES // B)
        c0 = hg * HPC * Dh
        csl = slice(c0, c0 + HPC * Dh)
        wqk_full = np.concatenate(
            [Wqkv[:, csl], Wqkv[:, D + c0: D + c0 + HPC * Dh]], axis=1
        ).astype(ml_dtypes.bfloat16)
        wv_full = Wqkv[:, 2 * D + c0: 2 * D + c0 + HPC * Dh].astype(
            ml_dtypes.bfloat16)
        in_maps.append({
            "consts": _consts(),
            "xT": np.ascontiguousarray(x[b].T).astype(ml_dtypes.bfloat16),
            "wqk": np.ascontiguousarray(
                wqk_full.reshape(KO, 128, 2 * HPC * Dh).transpose(1, 0, 2)),
            "wv": np.ascontiguousarray(
                wv_full.reshape(KO, 128, HPC * Dh).transpose(1, 0, 2)),
            "wout": np.ascontiguousarray(
                Wout[csl, :].astype(ml_dtypes.bfloat16)
                .reshape(2, 128, D).transpose(1, 0, 2)),
        })
    return in_maps


def kernel(x, causal_mask, key_padding_mask, Wqkv, bqkv, Wout, bout,
           _trace=False):
    from concourse.bass_utils import run_bass_kernel_spmd

    x = np.asarray(x, dtype=np.float32)
    Wqkv = np.asarray(Wqkv, dtype=np.float32)
    Wout = np.asarray(Wout, dtype=np.float32)
    bqkv = np.asarray(bqkv, dtype=np.float32)
    bout = np.asarray(bout, dtype=np.float32)
    if np.any(np.asarray(key_padding_mask)):
        raise NotImplementedError("key_padding_mask with padded keys")
    if np.any(bqkv):
        raise NotImplementedError("nonzero bqkv")

    nc = _get_program()
    in_maps = make_in_maps(x, Wqkv, Wout)
    res = run_bass_kernel_spmd(nc, in_maps, core_ids=list(range(NCORES)),
                               trace=_trace)
    G = NCORES // B
    outp = np.empty((B, T, D), dtype=np.float32)
    for b in range(B):
        acc = res.results[b * G]["out"].astype(np.float32)
        for hg in range(1, G):
            acc = acc + res.results[b * G + hg]["out"].astype(np.float32)
        outp[b] = acc + bout
    kernel.last_exec_time_ns = res.exec_time_ns
    return outp


# revision 12
# speedup vs baseline: 1.0722x; 1.0722x over previous
"""Multi-head self-attention (B=2, T=2048, D=1024, H=16) on 8 TRN2 NeuronCores.

Sharding: core c -> (b = c // 4, head-group hg = c % 4); each core computes the
full causal attention + partial output projection for its 4 heads of one batch
element.  The host pre-transposes x, pre-slices Wqkv columns / Wout rows per
head group, and sums the 4 partial projections per batch element (+ bout).

Device-side dataflow (per core), all matmuls bf16:
  A) qkT[c,t] = W[:,c].T @ xT  (c-major, chains of 8 contraction matmuls per
     (cb, it) psum tile; cb order q0,k0,q1,k1 so pair-0 attention can start
     as early as possible)
     V[t,c] = xT[:,t].T @ Wv  (+ones columns for row sums)
  B) it-major attention: S^T[j,i] = kT.T @ qT per (it, jb, pair) with the two
     heads of a pair row-packed (lhsT partitions 0:64 / 64:128); diagonal
     blocks only compute columns >= 128q (exact causal).
     P^T = exp(S^T/8) on ScalarE straight out of PSUM (both heads per call).
     ctx^T (+replicated sums rows) = [V|1].T @ P^T accumulated per pair.
     finish: DVE reciprocal on the 64-replicated sums rows straight from
     PSUM, one SBUF->SBUF partition-shift DMA, normalize on DVE.  ScalarE
     does nothing but exp in phase B.
  C) out[t,e] = ctx^T.T @ Wout_shard, interleaved per-it into phase B
     (shares the ps_s PSUM rotation), bf16 partial result back to host.
"""

import math
from contextlib import ExitStack

import numpy as np
import ml_dtypes

import concourse.bass as bass
import concourse.bacc as bacc_mod
import concourse.mybir as mybir
import concourse.tile as tile

FP32 = mybir.dt.float32
BF16 = mybir.dt.bfloat16
AF = mybir.ActivationFunctionType
ALU = mybir.AluOpType

B, T, D, H = 2, 2048, 1024, 16
Dh = D // H          # 64
NCORES = 8
HPC = 4              # heads per core
NPAIR = HPC // 2     # head pairs (2 heads share a 128-partition block)
IT = T // 512        # 4 query tiles of 512
JB = T // 128        # 16 key blocks of 128
KO = D // 128        # 8 contraction blocks for the projections
SCALE = 1.0 / math.sqrt(Dh)


def build_program(compile=True):
    nc = bacc_mod.Bacc()

    xT = nc.declare_dram_parameter("xT", [D, T], BF16, isOutput=False)
    wqk = nc.declare_dram_parameter("wqk", [128, KO, 2 * HPC * Dh], BF16,
                                    isOutput=False)
    wv = nc.declare_dram_parameter("wv", [128, KO, HPC * Dh], BF16,
                                   isOutput=False)
    wout = nc.declare_dram_parameter("wout", [128, 2, D], BF16, isOutput=False)
    # consts: [tri 128 | ones-col 64 | zeros 384]
    consts = nc.declare_dram_parameter("consts", [128, 576], BF16, isOutput=False)
    out = nc.declare_dram_parameter("out", [T, D], BF16, isOutput=True)

    xT_r = xT.rearrange("(o p) t -> p o t", p=128)

    with ExitStack() as ctx:
        tc = ctx.enter_context(tile.TileContext(nc))
        persist = ctx.enter_context(tc.tile_pool(name="persist", bufs=1))

        # ---------------- persistent tiles ----------------
        qkT = {}
        for nm in ("qT0", "kT0", "qT1", "kT1"):
            qkT[nm] = persist.tile([128, T], BF16, name=nm, tag=nm)
        V_aug = persist.tile([128, JB, HPC, 128], BF16, name="V_aug", tag="V_aug")
        merged = [
            persist.tile([128, IT, 512], BF16, name=f"merged{p}", tag=f"merged{p}")
            for p in range(NPAIR)
        ]
        wout_sb = persist.tile([128, 2, D], BF16, name="wout_sb", tag="wout_sb")
        consts_sb = persist.tile([128, 576], BF16, name="consts_sb",
                                 tag="consts_sb")
        tri = consts_sb[:, 0:128]

        diag_pT = {
            (q, pr): persist.tile([128, 2, 512], BF16, name=f"pTd{q}_{pr}",
                                  tag=f"pTd{q}_{pr}")
            for q in range(4) for pr in range(NPAIR)
        }

        def load_consts():
            # ones columns 64..127 of V_aug weights: the AV matmul then emits
            # the softmax denominators replicated on PSUM rows 64..127
            nc.vector.tensor_copy(
                V_aug[:, :, :, 64:],
                consts_sb[:, None, None, 128:192].to_broadcast(
                    (128, JB, HPC, 64)),
            )
            # pre-zero the fully-masked column prefix [0, 128q) of diagonal
            # P^T pair-tiles (exp only ever writes columns >= 128q; the
            # triangle multiply covers the square)
            for (q, pr), t_ in diag_pT.items():
                if q > 0:
                    for hl in range(2):
                        nc.vector.tensor_copy(
                            t_[:, hl, : 128 * q],
                            consts_sb[:, 192: 192 + 128 * q],
                        )

        # ---------------- phase A: QKV projections ----------------
        with (
            tc.tile_pool(name="phA", bufs=1) as pa,
            tc.tile_pool(name="psA", bufs=1, space="PSUM") as psa,
        ):
            xT_sb = pa.tile([128, KO, T], BF16, name="xT_sb", tag="xT_sb", bufs=1)
            wqk_sb = pa.tile([128, KO, 2 * HPC * Dh], BF16, name="wqk_sb",
                             tag="wqk_sb", bufs=1)
            wv_sb = pa.tile([128, KO, HPC * Dh], BF16, name="wv_sb", tag="wv_sb",
                            bufs=1)
            # interleave weight/x DMAs per o-block, it0 chunks first, so the
            # first matmul chain can start within ~2us
            for o in range(KO):
                nc.sync.dma_start(wqk_sb[:, o], wqk[:, o])
                nc.sync.dma_start(xT_sb[:, o, 0:512], xT_r[:, o, 0:512])
            nc.sync.dma_start(consts_sb[:], consts[:])
            load_consts()
            for it in range(1, IT):
                isl = slice(512 * it, 512 * (it + 1))
                for o in range(KO):
                    nc.sync.dma_start(xT_sb[:, o, isl], xT_r[:, o, isl])
            nc.sync.dma_start(wv_sb[:], wv[:])
            nc.sync.dma_start(wout_sb[:], wout[:])

            # qT/kT: [c, t] c-major; per (cb, it): one 8-matmul contraction
            # chain into one psum bank (LDWEIGHTS hides inside the chain)
            dests = [qkT["qT0"], qkT["kT0"], qkT["qT1"], qkT["kT1"]]
            cbs = [0, 2, 1, 3]  # wqk column blocks: q0, k0, q1, k1
            for di, cb in enumerate(cbs):
                for it in range(IT):
                    isl = slice(512 * it, 512 * (it + 1))
                    ps = psa.tile([128, 512], FP32, name="ps_a", tag="ps_a",
                                  bufs=3)
                    for o in range(KO):
                        nc.tensor.matmul(
                            ps[:],
                            lhsT=wqk_sb[:, o, 128 * cb: 128 * (cb + 1)],
                            rhs=xT_sb[:, o, isl],
                            start=(o == 0), stop=(o == KO - 1),
                        )
                    eng = nc.scalar if it % 2 == 0 else nc.vector
                    if eng is nc.scalar:
                        nc.scalar.copy(dests[di][:, isl], ps[:])
                    else:
                        nc.vector.tensor_copy(dests[di][:, isl], ps[:])

            # V natural [t, c] -> V_aug[:, tb, h, 0:64]
            for tb in range(JB):
                psv = psa.tile([128, HPC * Dh], FP32, name="ps_v", tag="ps_v",
                               bufs=2)
                for o in range(KO):
                    nc.tensor.matmul(
                        psv[:],
                        lhsT=xT_sb[:, o, 128 * tb: 128 * (tb + 1)],
                        rhs=wv_sb[:, o],
                        start=(o == 0), stop=(o == KO - 1),
                    )
                nc.vector.tensor_copy(
                    V_aug[:, tb, :, 0:64],
                    psv[:].rearrange("p (h d) -> p h d", h=HPC),
                )

        # ---------------- phase B: attention (+ phase C interleaved) --------
        with (
            tc.tile_pool(name="phB", bufs=2) as pb,
            tc.tile_pool(name="psB", bufs=1, space="PSUM") as psb,
        ):
            merged_flat = [m.rearrange("p a b -> p (a b)") for m in merged]

            def finish_pair(it, pair, psum_ctx):
                """Drain one pair's ctx^T and normalize, ScalarE-free.

                psum_ctx rows 0:64   = ctx^T (hl 0 = head even, 1 = odd)
                          rows 64:128 = sums replicated x64 per hl.
                """
                # full-partition reciprocal: rows 64:128 become 1/sums, rows
                # 0:64 are garbage (1/ctx) and never read
                recs = pb.tile([128, 2, 512], FP32, name="recs", tag="recs",
                               bufs=2)
                nc.vector.reciprocal_approx_fast(recs[:], psum_ctx[:])
                # unnormalized ctx out of PSUM (frees the accumulator fast)
                ctxU = pb.tile([64, 2, 512], BF16, name="ctxU", tag="ctxU",
                               bufs=2)
                nc.vector.tensor_copy(ctxU[:], psum_ctx[0:64, :, :])
                # shift recips to partitions 0:64 where ctx lives
                recsLo = pb.tile([64, 2, 512], FP32, name="recsLo",
                                 tag="recsLo", bufs=2)
                nc.sync.dma_start(recsLo[:], recs[64:128, :, :])
                # normalize + pack [h0 | h1] into merged partitions
                nc.vector.tensor_tensor(
                    out=merged[pair][0:64, it], in0=ctxU[:, 0, :],
                    in1=recsLo[:, 0, :], op=ALU.mult,
                )
                tmp = pb.tile([64, 512], BF16, name="odd_tmp", tag="odd_tmp",
                              bufs=2)
                nc.vector.tensor_tensor(
                    out=tmp[:], in0=ctxU[:, 1, :], in1=recsLo[:, 1, :],
                    op=ALU.mult,
                )
                nc.sync.dma_start(merged[pair][64:128, it], tmp[:])

            def phase_c_chunk_tb(it, k):
                """Output projection for t-block 4it+k (shares the ps_s PSUM
                rotation; emitted spread through the NEXT it's jb loop so it
                fills exp-wait gaps instead of serializing)."""
                tb = 4 * it + k
                tsl = slice(128 * tb, 128 * (tb + 1))
                pso = psb.tile([128, 2, 512], FP32, name="ps_o",
                               tag="ps_s", bufs=3)
                for pair in range(NPAIR):
                    for et in range(2):
                        nc.tensor.matmul(
                            pso[:, et, :],
                            lhsT=merged_flat[pair][:, tsl],
                            rhs=wout_sb[:, pair, 512 * et: 512 * (et + 1)],
                            start=(pair == 0), stop=(pair == NPAIR - 1),
                        )
                osb = pb.tile([128, 2, 512], BF16, name="osb", tag="osb",
                              bufs=3)
                nc.vector.tensor_copy(osb[:], pso[:])
                nc.sync.dma_start(
                    out[tsl, :], osb.rearrange("p a b -> p (a b)"))

            for it in range(IT):
                isl = slice(512 * it, 512 * (it + 1))
                njb = 4 * it + 4  # causal: j blocks 0 .. 4it+3
                # diagonal j-blocks first: the q=0 block opens the ctx
                # accumulation full-width, later diag blocks accumulate only
                # their causal column range, and the last full block closes
                # the group full-width (it=0 has no full blocks -> no trim)
                jb_order = list(range(4 * it, njb)) + list(range(4 * it))
                trim_av = it > 0
                # pair-sequential: one ctx accumulator live at a time, which
                # frees PSUM for a 3-deep ps_s rotation (QK runs ahead of exp)
                for pair in range(NPAIR):
                    kT_t = qkT[f"kT{pair}"]
                    qT_t = qkT[f"qT{pair}"]
                    psum_ctx = psb.tile([128, 2, 512], FP32, name="psum_ctx",
                                        tag="psum_ctx", bufs=1)
                    for ji, jb in enumerate(jb_order):
                        if it > 0 and ji >= 2 and ji - 2 < 2:
                            phase_c_chunk_tb(it - 1, 2 * pair + ji - 2)
                        jsl = slice(128 * jb, 128 * (jb + 1))
                        q = jb - 4 * it
                        c0 = 128 * q if q > 0 else 0  # exact-causal col start
                        av0 = c0 if trim_av else 0
                        ps2 = psb.tile([128, 2, 512], FP32, name="ps_s",
                                       tag="ps_s", bufs=3)
                        # two heads row-packed (lhsT partitions 0:64 / 64:128)
                        for hl in range(2):
                            rows = slice(64 * hl, 64 * (hl + 1))
                            nc.tensor.matmul(
                                ps2[:, hl, c0:],
                                lhsT=kT_t[rows, jsl],
                                rhs=qT_t[rows, isl][:, c0:],
                                start=True, stop=True,
                            )
                        if q < 0:  # fully sub-diagonal block: plain exp
                            pT = pb.tile([128, 2, 512], BF16, name="pT",
                                         tag="pT_full", bufs=3)
                            nc.scalar.activation(pT[:], ps2[:], AF.Exp,
                                                 scale=SCALE)
                        else:      # diagonal-class block
                            pT = diag_pT[(q, pair)]
                            nc.scalar.activation(
                                pT[:, :, c0:], ps2[:, :, c0:],
                                AF.Exp, scale=SCALE,
                            )
                            for hl in range(2):
                                nc.vector.tensor_tensor(
                                    out=pT[:, hl, 128 * q: 128 * (q + 1)],
                                    in0=pT[:, hl, 128 * q: 128 * (q + 1)],
                                    in1=tri[:],
                                    op=ALU.mult,
                                )
                        for hl in range(2):
                            h = 2 * pair + hl
                            nc.tensor.matmul(
                                psum_ctx[:, hl, av0:],
                                lhsT=V_aug[:, jb, h, :],
                                rhs=pT[:, hl, av0:],
                                start=(ji == 0), stop=(ji == njb - 1),
                            )
                    finish_pair(it, pair, psum_ctx)
            for k in range(4):
                phase_c_chunk_tb(IT - 1, k)

    if compile:
        nc.compile()
    return nc


_PROGRAM = None


def _get_program():
    global _PROGRAM
    if _PROGRAM is None:
        _PROGRAM = build_program()
    return _PROGRAM


def _consts():
    c = np.zeros((128, 576), ml_dtypes.bfloat16)
    dj = np.arange(128)[:, None]
    di = np.arange(128)[None, :]
    c[:, 0:128] = (dj <= di).astype(ml_dtypes.bfloat16)   # causal triangle
    c[:, 128:192] = 1.0                      # 64 ones columns
    return c


def make_in_maps(x, Wqkv, Wout):
    in_maps = []
    for core in range(NCORES):
        b, hg = core // (NCORES // B), core % (NCORES // B)
        c0 = hg * HPC * Dh
        csl = slice(c0, c0 + HPC * Dh)
        wqk_full = np.concatenate(
            [Wqkv[:, csl], Wqkv[:, D + c0: D + c0 + HPC * Dh]], axis=1
        ).astype(ml_dtypes.bfloat16)
        wv_full = Wqkv[:, 2 * D + c0: 2 * D + c0 + HPC * Dh].astype(
            ml_dtypes.bfloat16)
        in_maps.append({
            "consts": _consts(),
            "xT": np.ascontiguousarray(x[b].T).astype(ml_dtypes.bfloat16),
            "wqk": np.ascontiguousarray(
                wqk_full.reshape(KO, 128, 2 * HPC * Dh).transpose(1, 0, 2)),
            "wv": np.ascontiguousarray(
                wv_full.reshape(KO, 128, HPC * Dh).transpose(1, 0, 2)),
            "wout": np.ascontiguousarray(
                Wout[csl, :].astype(ml_dtypes.bfloat16)
                .reshape(2, 128, D).transpose(1, 0, 2)),
        })
    return in_maps


def kernel(x, causal_mask, key_padding_mask, Wqkv, bqkv, Wout, bout,
           _trace=False):
    from concourse.bass_utils import run_bass_kernel_spmd

    x = np.asarray(x, dtype=np.float32)
    Wqkv = np.asarray(Wqkv, dtype=np.float32)
    Wout = np.asarray(Wout, dtype=np.float32)
    bqkv = np.asarray(bqkv, dtype=np.float32)
    bout = np.asarray(bout, dtype=np.float32)
    if np.any(np.asarray(key_padding_mask)):
        raise NotImplementedError("key_padding_mask with padded keys")
    if np.any(bqkv):
        raise NotImplementedError("nonzero bqkv")

    nc = _get_program()
    in_maps = make_in_maps(x, Wqkv, Wout)
    res = run_bass_kernel_spmd(nc, in_maps, core_ids=list(range(NCORES)),
                               trace=_trace)
    G = NCORES // B
    outp = np.empty((B, T, D), dtype=np.float32)
    for b in range(B):
        acc = res.results[b * G]["out"].astype(np.float32)
        for hg in range(1, G):
            acc = acc + res.results[b * G + hg]["out"].astype(np.float32)
        outp[b] = acc + bout
    kernel.last_exec_time_ns = res.exec_time_ns
    return outp


# revision 16
# speedup vs baseline: 1.1027x; 1.0285x over previous
"""Multi-head self-attention (B=2, T=2048, D=1024, H=16) on 8 TRN2 NeuronCores.

Sharding: core c -> (b = c // 4, head-group hg = c % 4); each core computes the
full causal attention + partial output projection for its 4 heads of one batch
element.  The host pre-transposes x, pre-slices Wqkv columns / Wout rows per
head group, and sums the 4 partial projections per batch element (+ bout).

Device-side dataflow (per core), all matmuls bf16:
  A) qkT[c,t] = W[:,c].T @ xT  (c-major, chains of 8 contraction matmuls per
     (cb, it) psum tile; cb order q0,k0,q1,k1 so pair-0 attention can start
     as early as possible)
     V[t,c] = xT[:,t].T @ Wv  (+ones columns for row sums)
  B) it-major attention: S^T[j,i] = kT.T @ qT per (it, jb, pair) with the two
     heads of a pair row-packed (lhsT partitions 0:64 / 64:128); diagonal
     blocks only compute columns >= 128q (exact causal).
     P^T = exp(S^T/8) on ScalarE straight out of PSUM (both heads per call).
     ctx^T (+replicated sums rows) = [V|1].T @ P^T accumulated per pair.
     finish: DVE reciprocal on the 64-replicated sums rows straight from
     PSUM, one SBUF->SBUF partition-shift DMA, normalize on DVE.  ScalarE
     does nothing but exp in phase B.
  C) out[t,e] = ctx^T.T @ Wout_shard, interleaved per-it into phase B
     (shares the ps_s PSUM rotation), bf16 partial result back to host.
"""

import math
from contextlib import ExitStack

import numpy as np
import ml_dtypes

import concourse.bass as bass
import concourse.bacc as bacc_mod
import concourse.mybir as mybir
import concourse.tile as tile

FP32 = mybir.dt.float32
BF16 = mybir.dt.bfloat16
AF = mybir.ActivationFunctionType
ALU = mybir.AluOpType

B, T, D, H = 2, 2048, 1024, 16
Dh = D // H          # 64
NCORES = 8
HPC = 4              # heads per core
NPAIR = HPC // 2     # head pairs (2 heads share a 128-partition block)
IT = T // 512        # 4 query tiles of 512
JB = T // 128        # 16 key blocks of 128
KO = D // 128        # 8 contraction blocks for the projections
SCALE = 1.0 / math.sqrt(Dh)


def build_program(compile=True):
    nc = bacc_mod.Bacc()

    xT = nc.declare_dram_parameter("xT", [D, T], BF16, isOutput=False)
    wqk = nc.declare_dram_parameter("wqk", [128, KO, 2 * HPC * Dh], BF16,
                                    isOutput=False)
    wv = nc.declare_dram_parameter("wv", [128, KO, HPC * Dh], BF16,
                                   isOutput=False)
    wout = nc.declare_dram_parameter("wout", [128, 2, D], BF16, isOutput=False)
    # consts: [tri 128 | ones-col 64 | zeros 384]
    consts = nc.declare_dram_parameter("consts", [128, 576], BF16, isOutput=False)
    out = nc.declare_dram_parameter("out", [T, D], BF16, isOutput=True)

    xT_r = xT.rearrange("(o p) t -> p o t", p=128)

    with ExitStack() as ctx:
        tc = ctx.enter_context(tile.TileContext(nc))
        persist = ctx.enter_context(tc.tile_pool(name="persist", bufs=1))

        # ---------------- persistent tiles ----------------
        qkT = {}
        for nm in ("qT0", "kT0", "qT1", "kT1"):
            qkT[nm] = persist.tile([128, T], BF16, name=nm, tag=nm)
        V_aug = persist.tile([128, JB, HPC, 128], BF16, name="V_aug", tag="V_aug")
        merged = [
            persist.tile([128, IT, 512], BF16, name=f"merged{p}", tag=f"merged{p}")
            for p in range(NPAIR)
        ]
        wout_sb = persist.tile([128, 2, D], BF16, name="wout_sb", tag="wout_sb")
        consts_sb = persist.tile([128, 576], BF16, name="consts_sb",
                                 tag="consts_sb")
        tri = consts_sb[:, 0:128]

        diag_pT = {
            (q, pr): persist.tile([128, 2, 512], BF16, name=f"pTd{q}_{pr}",
                                  tag=f"pTd{q}_{pr}")
            for q in range(4) for pr in range(NPAIR)
        }

        def load_consts():
            # ones columns 64..127 of V_aug weights: the AV matmul then emits
            # the softmax denominators replicated on PSUM rows 64..127
            nc.vector.tensor_copy(
                V_aug[:, :, :, 64:],
                consts_sb[:, None, None, 128:192].to_broadcast(
                    (128, JB, HPC, 64)),
            )
            # pre-zero the fully-masked column prefix [0, 128q) of diagonal
            # P^T pair-tiles (exp only ever writes columns >= 128q; the
            # triangle multiply covers the square)
            for (q, pr), t_ in diag_pT.items():
                if q > 0:
                    for hl in range(2):
                        nc.vector.tensor_copy(
                            t_[:, hl, : 128 * q],
                            consts_sb[:, 192: 192 + 128 * q],
                        )

        # ---------------- phase A: QKV projections ----------------
        with (
            tc.tile_pool(name="phA", bufs=1) as pa,
            tc.tile_pool(name="psA", bufs=1, space="PSUM") as psa,
        ):
            xT_sb = pa.tile([128, KO, T], BF16, name="xT_sb", tag="xT_sb", bufs=1)
            wqk_sb = pa.tile([128, KO, 2 * HPC * Dh], BF16, name="wqk_sb",
                             tag="wqk_sb", bufs=1)
            wv_sb = pa.tile([128, KO, HPC * Dh], BF16, name="wv_sb", tag="wv_sb",
                            bufs=1)
            # interleave weight/x DMAs per o-block, it0 chunks first, so the
            # first matmul chain can start within ~2us
            for o in range(KO):
                nc.sync.dma_start(wqk_sb[:, o], wqk[:, o])
                nc.sync.dma_start(xT_sb[:, o], xT_r[:, o])
            nc.sync.dma_start(consts_sb[:], consts[:])
            load_consts()
            nc.sync.dma_start(wv_sb[:], wv[:])
            nc.sync.dma_start(wout_sb[:], wout[:])

            # qT/kT: [c, t] c-major; per (cb, it): one 8-matmul contraction
            # chain into one psum bank (LDWEIGHTS hides inside the chain)
            dests = [qkT["qT0"], qkT["kT0"], qkT["qT1"], qkT["kT1"]]
            cbs = [0, 2, 1, 3]  # wqk column blocks: q0, k0, q1, k1
            for di, cb in enumerate(cbs):
                for it in range(IT):
                    isl = slice(512 * it, 512 * (it + 1))
                    ps = psa.tile([128, 512], FP32, name="ps_a", tag="ps_a",
                                  bufs=3)
                    for o in range(KO):
                        nc.tensor.matmul(
                            ps[:],
                            lhsT=wqk_sb[:, o, 128 * cb: 128 * (cb + 1)],
                            rhs=xT_sb[:, o, isl],
                            start=(o == 0), stop=(o == KO - 1),
                        )
                    eng = nc.scalar if it % 2 == 0 else nc.vector
                    if eng is nc.scalar:
                        nc.scalar.copy(dests[di][:, isl], ps[:])
                    else:
                        nc.vector.tensor_copy(dests[di][:, isl], ps[:])

            # V natural [t, c] -> V_aug[:, tb, h, 0:64]
            for tb in range(JB):
                psv = psa.tile([128, HPC * Dh], FP32, name="ps_v", tag="ps_v",
                               bufs=2)
                for o in range(KO):
                    nc.tensor.matmul(
                        psv[:],
                        lhsT=xT_sb[:, o, 128 * tb: 128 * (tb + 1)],
                        rhs=wv_sb[:, o],
                        start=(o == 0), stop=(o == KO - 1),
                    )
                nc.vector.tensor_copy(
                    V_aug[:, tb, :, 0:64],
                    psv[:].rearrange("p (h d) -> p h d", h=HPC),
                )

        # ---------------- phase B: attention (+ phase C interleaved) --------
        with (
            tc.tile_pool(name="phB", bufs=2) as pb,
            tc.tile_pool(name="psB", bufs=1, space="PSUM") as psb,
        ):
            merged_flat = [m.rearrange("p a b -> p (a b)") for m in merged]

            def finish_pair(it, pair, psum_ctx):
                """Drain one pair's ctx^T and normalize, ScalarE-free.

                psum_ctx rows 0:64   = ctx^T (hl 0 = head even, 1 = odd)
                          rows 64:128 = sums replicated x64 per hl.
                """
                # full-partition reciprocal: rows 64:128 become 1/sums, rows
                # 0:64 are garbage (1/ctx) and never read
                recs = pb.tile([128, 2, 512], FP32, name="recs", tag="recs",
                               bufs=2)
                nc.vector.reciprocal_approx_fast(recs[:], psum_ctx[:])
                # unnormalized ctx out of PSUM (frees the accumulator fast)
                ctxU = pb.tile([64, 2, 512], BF16, name="ctxU", tag="ctxU",
                               bufs=2)
                nc.vector.tensor_copy(ctxU[:], psum_ctx[0:64, :, :])
                # shift recips to partitions 0:64 where ctx lives
                recsLo = pb.tile([64, 2, 512], FP32, name="recsLo",
                                 tag="recsLo", bufs=2)
                nc.sync.dma_start(recsLo[:], recs[64:128, :, :])
                # normalize + pack [h0 | h1] into merged partitions
                nc.vector.tensor_tensor(
                    out=merged[pair][0:64, it], in0=ctxU[:, 0, :],
                    in1=recsLo[:, 0, :], op=ALU.mult,
                )
                tmp = pb.tile([64, 512], BF16, name="odd_tmp", tag="odd_tmp",
                              bufs=2)
                nc.vector.tensor_tensor(
                    out=tmp[:], in0=ctxU[:, 1, :], in1=recsLo[:, 1, :],
                    op=ALU.mult,
                )
                nc.sync.dma_start(merged[pair][64:128, it], tmp[:])

            def phase_c_chunk_tb(it, k):
                """Output projection for t-block 4it+k (shares the ps_s PSUM
                rotation; emitted spread through the NEXT it's jb loop so it
                fills exp-wait gaps instead of serializing)."""
                tb = 4 * it + k
                tsl = slice(128 * tb, 128 * (tb + 1))
                pso = psb.tile([128, 2, 512], FP32, name="ps_o",
                               tag="ps_s", bufs=3)
                for pair in range(NPAIR):
                    for et in range(2):
                        nc.tensor.matmul(
                            pso[:, et, :],
                            lhsT=merged_flat[pair][:, tsl],
                            rhs=wout_sb[:, pair, 512 * et: 512 * (et + 1)],
                            start=(pair == 0), stop=(pair == NPAIR - 1),
                        )
                osb = pb.tile([128, 2, 512], BF16, name="osb", tag="osb",
                              bufs=3)
                nc.vector.tensor_copy(osb[:], pso[:])
                nc.sync.dma_start(
                    out[tsl, :], osb.rearrange("p a b -> p (a b)"))

            it_order = list(range(IT - 1, -1, -1))  # big it first: its
            # finish/output-projection hides under the next stream; the
            # exposed tail belongs to the smallest it
            for io, it in enumerate(it_order):
                prev_it = it_order[io - 1] if io > 0 else None
                isl = slice(512 * it, 512 * (it + 1))
                njb = 4 * it + 4  # causal: j blocks 0 .. 4it+3
                # diagonal j-blocks first: the q=0 block opens the ctx
                # accumulation full-width, later diag blocks accumulate only
                # their causal column range, and the last full block closes
                # the group full-width (it=0 has no full blocks -> no trim)
                jb_order = list(range(4 * it, njb)) + list(range(4 * it))
                trim_av = it > 0
                # pair-sequential: one ctx accumulator live at a time, which
                # frees PSUM for a 3-deep ps_s rotation (QK runs ahead of exp)
                for pair in range(NPAIR):
                    kT_t = qkT[f"kT{pair}"]
                    qT_t = qkT[f"qT{pair}"]
                    psum_ctx = psb.tile([128, 2, 512], FP32, name="psum_ctx",
                                        tag="psum_ctx", bufs=1)
                    for ji, jb in enumerate(jb_order):
                        if prev_it is not None and ji >= 1 and ji - 1 < 2:
                            phase_c_chunk_tb(prev_it, 2 * pair + ji - 1)
                        jsl = slice(128 * jb, 128 * (jb + 1))
                        q = jb - 4 * it
                        c0 = 128 * q if q > 0 else 0  # exact-causal col start
                        av0 = c0 if trim_av else 0
                        ps2 = psb.tile([128, 2, 512], FP32, name="ps_s",
                                       tag="ps_s", bufs=3)
                        # two heads row-packed (lhsT partitions 0:64 / 64:128)
                        for hl in range(2):
                            rows = slice(64 * hl, 64 * (hl + 1))
                            nc.tensor.matmul(
                                ps2[:, hl, c0:],
                                lhsT=kT_t[rows, jsl],
                                rhs=qT_t[rows, isl][:, c0:],
                                start=True, stop=True,
                            )
                        if q < 0:  # fully sub-diagonal block: plain exp
                            pT = pb.tile([128, 2, 512], BF16, name="pT",
                                         tag="pT_full", bufs=3)
                            nc.scalar.activation(pT[:], ps2[:], AF.Exp,
                                                 scale=SCALE)
                        else:      # diagonal-class block
                            pT = diag_pT[(q, pair)]
                            nc.scalar.activation(
                                pT[:, :, c0:], ps2[:, :, c0:],
                                AF.Exp, scale=SCALE,
                            )
                            for hl in range(2):
                                nc.vector.tensor_tensor(
                                    out=pT[:, hl, 128 * q: 128 * (q + 1)],
                                    in0=pT[:, hl, 128 * q: 128 * (q + 1)],
                                    in1=tri[:],
                                    op=ALU.mult,
                                )
                        for hl in range(2):
                            h = 2 * pair + hl
                            nc.tensor.matmul(
                                psum_ctx[:, hl, av0:],
                                lhsT=V_aug[:, jb, h, :],
                                rhs=pT[:, hl, av0:],
                                start=(ji == 0), stop=(ji == njb - 1),
                            )
                    finish_pair(it, pair, psum_ctx)
            for k in range(4):
                phase_c_chunk_tb(it_order[-1], k)

    if compile:
        nc.compile()
    return nc


_PROGRAM = None


def _get_program():
    global _PROGRAM
    if _PROGRAM is None:
        _PROGRAM = build_program()
    return _PROGRAM


def _consts():
    c = np.zeros((128, 576), ml_dtypes.bfloat16)
    dj = np.arange(128)[:, None]
    di = np.arange(128)[None, :]
    c[:, 0:128] = (dj <= di).astype(ml_dtypes.bfloat16)   # causal triangle
    c[:, 128:192] = 1.0                      # 64 ones columns
    return c


def make_in_maps(x, Wqkv, Wout):
    in_maps = []
    for core in range(NCORES):
        b, hg = core // (NCORES // B), core % (NCORES // B)
        c0 = hg * HPC * Dh
        csl = slice(c0, c0 + HPC * Dh)
        wqk_full = np.concatenate(
            [Wqkv[:, csl], Wqkv[:, D + c0: D + c0 + HPC * Dh]], axis=1
        ).astype(ml_dtypes.bfloat16)
        wv_full = Wqkv[:, 2 * D + c0: 2 * D + c0 + HPC * Dh].astype(
            ml_dtypes.bfloat16)
        in_maps.append({
            "consts": _consts(),
            "xT": np.ascontiguousarray(x[b].T).astype(ml_dtypes.bfloat16),
            "wqk": np.ascontiguousarray(
                wqk_full.reshape(KO, 128, 2 * HPC * Dh).transpose(1, 0, 2)),
            "wv": np.ascontiguousarray(
                wv_full.reshape(KO, 128, HPC * Dh).transpose(1, 0, 2)),
            "wout": np.ascontiguousarray(
                Wout[csl, :].astype(ml_dtypes.bfloat16)
                .reshape(2, 128, D).transpose(1, 0, 2)),
        })
    return in_maps


def kernel(x, causal_mask, key_padding_mask, Wqkv, bqkv, Wout, bout,
           _trace=False):
    from concourse.bass_utils import run_bass_kernel_spmd

    x = np.asarray(x, dtype=np.float32)
    Wqkv = np.asarray(Wqkv, dtype=np.float32)
    Wout = np.asarray(Wout, dtype=np.float32)
    bqkv = np.asarray(bqkv, dtype=np.float32)
    bout = np.asarray(bout, dtype=np.float32)
    if np.any(np.asarray(key_padding_mask)):
        raise NotImplementedError("key_padding_mask with padded keys")
    if np.any(bqkv):
        raise NotImplementedError("nonzero bqkv")

    nc = _get_program()
    in_maps = make_in_maps(x, Wqkv, Wout)
    res = run_bass_kernel_spmd(nc, in_maps, core_ids=list(range(NCORES)),
                               trace=_trace)
    G = NCORES // B
    outp = np.empty((B, T, D), dtype=np.float32)
    for b in range(B):
        acc = res.results[b * G]["out"].astype(np.float32)
        for hg in range(1, G):
            acc = acc + res.results[b * G + hg]["out"].astype(np.float32)
        outp[b] = acc + bout
    kernel.last_exec_time_ns = res.exec_time_ns
    return outp


# revision 17
# speedup vs baseline: 1.1305x; 1.0252x over previous
"""Multi-head self-attention (B=2, T=2048, D=1024, H=16) on 8 TRN2 NeuronCores.

Sharding: core c -> (b = c // 4, head-group hg = c % 4); each core computes the
full causal attention + partial output projection for its 4 heads of one batch
element.  The host pre-transposes x, pre-slices Wqkv columns / Wout rows per
head group, and sums the 4 partial projections per batch element (+ bout).

Device-side dataflow (per core), all matmuls bf16:
  A) qkT[c,t] = W[:,c].T @ xT  (c-major, chains of 8 contraction matmuls per
     (cb, it) psum tile; cb order q0,k0,q1,k1 so pair-0 attention can start
     as early as possible)
     V[t,c] = xT[:,t].T @ Wv  (+ones columns for row sums)
  B) it-major attention: S^T[j,i] = kT.T @ qT per (it, jb, pair) with the two
     heads of a pair row-packed (lhsT partitions 0:64 / 64:128); diagonal
     blocks only compute columns >= 128q (exact causal).
     P^T = exp(S^T/8) on ScalarE straight out of PSUM (both heads per call).
     ctx^T (+replicated sums rows) = [V|1].T @ P^T accumulated per pair.
     finish: DVE reciprocal on the 64-replicated sums rows straight from
     PSUM, one SBUF->SBUF partition-shift DMA, normalize on DVE.  ScalarE
     does nothing but exp in phase B.
  C) out[t,e] = ctx^T.T @ Wout_shard, interleaved per-it into phase B
     (shares the ps_s PSUM rotation), bf16 partial result back to host.
"""

import math
from contextlib import ExitStack

import numpy as np
import ml_dtypes

import concourse.bass as bass
import concourse.bacc as bacc_mod
import concourse.mybir as mybir
import concourse.tile as tile

FP32 = mybir.dt.float32
BF16 = mybir.dt.bfloat16
AF = mybir.ActivationFunctionType
ALU = mybir.AluOpType

B, T, D, H = 2, 2048, 1024, 16
Dh = D // H          # 64
NCORES = 8
HPC = 4              # heads per core
NPAIR = HPC // 2     # head pairs (2 heads share a 128-partition block)
IT = T // 512        # 4 query tiles of 512
JB = T // 128        # 16 key blocks of 128
KO = D // 128        # 8 contraction blocks for the projections
SCALE = 1.0 / math.sqrt(Dh)


def build_program(compile=True):
    nc = bacc_mod.Bacc()

    xT = nc.declare_dram_parameter("xT", [D, T], BF16, isOutput=False)
    wqk = nc.declare_dram_parameter("wqk", [128, KO, 2 * HPC * Dh], BF16,
                                    isOutput=False)
    wv = nc.declare_dram_parameter("wv", [128, KO, HPC * Dh], BF16,
                                   isOutput=False)
    wout = nc.declare_dram_parameter("wout", [128, 2, D], BF16, isOutput=False)
    # consts: [tri 128 | ones-col 64 | zeros 384]
    consts = nc.declare_dram_parameter("consts", [128, 576], BF16, isOutput=False)
    out = nc.declare_dram_parameter("out", [T, D], BF16, isOutput=True)

    xT_r = xT.rearrange("(o p) t -> p o t", p=128)

    with ExitStack() as ctx:
        tc = ctx.enter_context(tile.TileContext(nc))
        persist = ctx.enter_context(tc.tile_pool(name="persist", bufs=1))

        # ---------------- persistent tiles ----------------
        qkT = {}
        for nm in ("qT0", "kT0", "qT1", "kT1"):
            qkT[nm] = persist.tile([128, T], BF16, name=nm, tag=nm)
        V_aug = persist.tile([128, JB, HPC, 128], BF16, name="V_aug", tag="V_aug")
        merged = [
            persist.tile([128, IT, 512], BF16, name=f"merged{p}", tag=f"merged{p}")
            for p in range(NPAIR)
        ]
        wout_sb = persist.tile([128, 2, D], BF16, name="wout_sb", tag="wout_sb")
        consts_sb = persist.tile([128, 576], BF16, name="consts_sb",
                                 tag="consts_sb")
        tri = consts_sb[:, 0:128]

        diag_pT = {
            (q, pr): persist.tile([128, 2, 512], BF16, name=f"pTd{q}_{pr}",
                                  tag=f"pTd{q}_{pr}")
            for q in range(4) for pr in range(NPAIR)
        }

        def load_consts():
            # ones columns 64..127 of V_aug weights: the AV matmul then emits
            # the softmax denominators replicated on PSUM rows 64..127
            nc.vector.tensor_copy(
                V_aug[:, :, :, 64:],
                consts_sb[:, None, None, 128:192].to_broadcast(
                    (128, JB, HPC, 64)),
            )
            # pre-zero the fully-masked column prefix [0, 128q) of diagonal
            # P^T pair-tiles (exp only ever writes columns >= 128q; the
            # triangle multiply covers the square)
            for (q, pr), t_ in diag_pT.items():
                if q > 0:
                    for hl in range(2):
                        nc.vector.tensor_copy(
                            t_[:, hl, : 128 * q],
                            consts_sb[:, 192: 192 + 128 * q],
                        )

        # ---------------- phase A: QKV projections ----------------
        with (
            tc.tile_pool(name="phA", bufs=1) as pa,
            tc.tile_pool(name="psA", bufs=1, space="PSUM") as psa,
        ):
            xT_sb = pa.tile([128, KO, T], BF16, name="xT_sb", tag="xT_sb", bufs=1)
            wqk_sb = pa.tile([128, KO, 2 * HPC * Dh], BF16, name="wqk_sb",
                             tag="wqk_sb", bufs=1)
            wv_sb = pa.tile([128, KO, HPC * Dh], BF16, name="wv_sb", tag="wv_sb",
                            bufs=1)
            # interleave weight/x DMAs per o-block, it0 chunks first, so the
            # first matmul chain can start within ~2us
            for o in range(KO):
                nc.sync.dma_start(wqk_sb[:, o], wqk[:, o])
                nc.sync.dma_start(xT_sb[:, o], xT_r[:, o])
            nc.sync.dma_start(consts_sb[:], consts[:])
            load_consts()
            nc.sync.dma_start(wv_sb[:], wv[:])
            nc.sync.dma_start(wout_sb[:], wout[:])

            # qT/kT: [c, t] c-major; per (cb, it): one 8-matmul contraction
            # chain into one psum bank (LDWEIGHTS hides inside the chain)
            dests = [qkT["qT0"], qkT["kT0"], qkT["qT1"], qkT["kT1"]]
            cbs = [0, 2, 1, 3]  # wqk column blocks: q0, k0, q1, k1
            for di, cb in enumerate(cbs):
                for it in range(IT):
                    isl = slice(512 * it, 512 * (it + 1))
                    ps = psa.tile([128, 512], FP32, name="ps_a", tag="ps_a",
                                  bufs=3)
                    for o in range(KO):
                        nc.tensor.matmul(
                            ps[:],
                            lhsT=wqk_sb[:, o, 128 * cb: 128 * (cb + 1)],
                            rhs=xT_sb[:, o, isl],
                            start=(o == 0), stop=(o == KO - 1),
                        )
                    eng = nc.scalar if it % 2 == 0 else nc.vector
                    if eng is nc.scalar:
                        nc.scalar.copy(dests[di][:, isl], ps[:])
                    else:
                        nc.vector.tensor_copy(dests[di][:, isl], ps[:])

            # V natural [t, c] -> V_aug[:, tb, h, 0:64]
            for tb in range(JB):
                psv = psa.tile([128, HPC * Dh], FP32, name="ps_v", tag="ps_v",
                               bufs=2)
                for o in range(KO):
                    nc.tensor.matmul(
                        psv[:],
                        lhsT=xT_sb[:, o, 128 * tb: 128 * (tb + 1)],
                        rhs=wv_sb[:, o],
                        start=(o == 0), stop=(o == KO - 1),
                    )
                nc.vector.tensor_copy(
                    V_aug[:, tb, :, 0:64],
                    psv[:].rearrange("p (h d) -> p h d", h=HPC),
                )

        # ---------------- phase B: attention (+ phase C interleaved) --------
        with (
            tc.tile_pool(name="phB", bufs=2) as pb,
            tc.tile_pool(name="psB", bufs=1, space="PSUM") as psb,
        ):
            merged_flat = [m.rearrange("p a b -> p (a b)") for m in merged]

            def finish_pair(it, pair, psum_ctx):
                """Drain one pair's ctx^T and normalize, ScalarE-free.

                psum_ctx rows 0:64   = ctx^T (hl 0 = head even, 1 = odd)
                          rows 64:128 = sums replicated x64 per hl.
                """
                # full-partition reciprocal: rows 64:128 become 1/sums, rows
                # 0:64 are garbage (1/ctx) and never read
                recs = pb.tile([128, 2, 512], FP32, name="recs", tag="recs",
                               bufs=2)
                nc.vector.reciprocal_approx_fast(recs[:], psum_ctx[:])
                # unnormalized ctx out of PSUM (frees the accumulator fast)
                ctxU = pb.tile([64, 2, 512], BF16, name="ctxU", tag="ctxU",
                               bufs=2)
                nc.vector.tensor_copy(ctxU[:], psum_ctx[0:64, :, :])
                # shift recips to partitions 0:64 where ctx lives
                recsLo = pb.tile([64, 2, 512], FP32, name="recsLo",
                                 tag="recsLo", bufs=2)
                nc.sync.dma_start(recsLo[:], recs[64:128, :, :])
                # normalize + pack [h0 | h1] into merged partitions
                nc.vector.tensor_tensor(
                    out=merged[pair][0:64, it], in0=ctxU[:, 0, :],
                    in1=recsLo[:, 0, :], op=ALU.mult,
                )
                tmp = pb.tile([64, 512], BF16, name="odd_tmp", tag="odd_tmp",
                              bufs=2)
                nc.vector.tensor_tensor(
                    out=tmp[:], in0=ctxU[:, 1, :], in1=recsLo[:, 1, :],
                    op=ALU.mult,
                )
                nc.sync.dma_start(merged[pair][64:128, it], tmp[:])

            def phase_c_chunk_tb(it, k):
                """Output projection for t-block 4it+k (shares the ps_s PSUM
                rotation; emitted spread through the NEXT it's jb loop so it
                fills exp-wait gaps instead of serializing)."""
                tb = 4 * it + k
                tsl = slice(128 * tb, 128 * (tb + 1))
                pso = psb.tile([128, 2, 512], FP32, name="ps_o",
                               tag="ps_s", bufs=3)
                for pair in range(NPAIR):
                    for et in range(2):
                        nc.tensor.matmul(
                            pso[:, et, :],
                            lhsT=merged_flat[pair][:, tsl],
                            rhs=wout_sb[:, pair, 512 * et: 512 * (et + 1)],
                            start=(pair == 0), stop=(pair == NPAIR - 1),
                        )
                osb = pb.tile([128, 2, 512], BF16, name="osb", tag="osb",
                              bufs=3)
                nc.vector.tensor_copy(osb[:], pso[:])
                nc.sync.dma_start(
                    out[tsl, :], osb.rearrange("p a b -> p (a b)"))

            it_order = list(range(IT - 1, -1, -1))  # big it first: its
            # finish/output-projection hides under the next stream; the
            # exposed tail belongs to the smallest it
            for io, it in enumerate(it_order):
                prev_it = it_order[io - 1] if io > 0 else None
                isl = slice(512 * it, 512 * (it + 1))
                njb = 4 * it + 4  # causal: j blocks 0 .. 4it+3
                # diagonal j-blocks first: the q=0 block opens the ctx
                # accumulation full-width, later diag blocks accumulate only
                # their causal column range, and the last full block closes
                # the group full-width (it=0 has no full blocks -> no trim)
                jb_order = list(range(4 * it, njb)) + list(range(4 * it))
                trim_av = it > 0
                # pair-sequential: one ctx accumulator live at a time, which
                # frees PSUM for a 3-deep ps_s rotation (QK runs ahead of exp)
                for pair in range(NPAIR):
                    kT_t = qkT[f"kT{pair}"]
                    qT_t = qkT[f"qT{pair}"]
                    psum_ctx = psb.tile([128, 2, 512], FP32, name="psum_ctx",
                                        tag="psum_ctx", bufs=1)
                    def emit_av(ji, jb, pT):
                        q = jb - 4 * it
                        av0 = (128 * q if q > 0 else 0) if trim_av else 0
                        for hl in range(2):
                            h = 2 * pair + hl
                            nc.tensor.matmul(
                                psum_ctx[:, hl, av0:],
                                lhsT=V_aug[:, jb, h, :],
                                rhs=pT[:, hl, av0:],
                                start=(ji == 0), stop=(ji == njb - 1),
                            )

                    pending_av = None  # 1-stage software pipeline: emit
                    # QK(k+1) on the in-order Tensor queue BEFORE AV(k), so
                    # QK(k+1) streams (and loads weights) during exp(k)
                    for ji, jb in enumerate(jb_order):
                        if prev_it is not None and ji >= 1 and ji - 1 < 2:
                            phase_c_chunk_tb(prev_it, 2 * pair + ji - 1)
                        jsl = slice(128 * jb, 128 * (jb + 1))
                        q = jb - 4 * it
                        c0 = 128 * q if q > 0 else 0  # exact-causal col start
                        ps2 = psb.tile([128, 2, 512], FP32, name="ps_s",
                                       tag="ps_s", bufs=3)
                        # two heads row-packed (lhsT partitions 0:64 / 64:128)
                        for hl in range(2):
                            rows = slice(64 * hl, 64 * (hl + 1))
                            nc.tensor.matmul(
                                ps2[:, hl, c0:],
                                lhsT=kT_t[rows, jsl],
                                rhs=qT_t[rows, isl][:, c0:],
                                start=True, stop=True,
                            )
                        if pending_av is not None:
                            emit_av(*pending_av)
                        if q < 0:  # fully sub-diagonal block: plain exp
                            pT = pb.tile([128, 2, 512], BF16, name="pT",
                                         tag="pT_full", bufs=4)
                            nc.scalar.activation(pT[:], ps2[:], AF.Exp,
                                                 scale=SCALE)
                        else:      # diagonal-class block
                            pT = diag_pT[(q, pair)]
                            nc.scalar.activation(
                                pT[:, :, c0:], ps2[:, :, c0:],
                                AF.Exp, scale=SCALE,
                            )
                            for hl in range(2):
                                nc.vector.tensor_tensor(
                                    out=pT[:, hl, 128 * q: 128 * (q + 1)],
                                    in0=pT[:, hl, 128 * q: 128 * (q + 1)],
                                    in1=tri[:],
                                    op=ALU.mult,
                                )
                        pending_av = (ji, jb, pT)
                    emit_av(*pending_av)
                    finish_pair(it, pair, psum_ctx)
            for k in range(4):
                phase_c_chunk_tb(it_order[-1], k)

    if compile:
        nc.compile()
    return nc


_PROGRAM = None


def _get_program():
    global _PROGRAM
    if _PROGRAM is None:
        _PROGRAM = build_program()
    return _PROGRAM


def _consts():
    c = np.zeros((128, 576), ml_dtypes.bfloat16)
    dj = np.arange(128)[:, None]
    di = np.arange(128)[None, :]
    c[:, 0:128] = (dj <= di).astype(ml_dtypes.bfloat16)   # causal triangle
    c[:, 128:192] = 1.0                      # 64 ones columns
    return c


def make_in_maps(x, Wqkv, Wout):
    in_maps = []
    for core in range(NCORES):
        b, hg = core // (NCORES // B), core % (NCORES // B)
        c0 = hg * HPC * Dh
        csl = slice(c0, c0 + HPC * Dh)
        wqk_full = np.concatenate(
            [Wqkv[:, csl], Wqkv[:, D + c0: D + c0 + HPC * Dh]], axis=1
        ).astype(ml_dtypes.bfloat16)
        wv_full = Wqkv[:, 2 * D + c0: 2 * D + c0 + HPC * Dh].astype(
            ml_dtypes.bfloat16)
        in_maps.append({
            "consts": _consts(),
            "xT": np.ascontiguousarray(x[b].T).astype(ml_dtypes.bfloat16),
            "wqk": np.ascontiguousarray(
                wqk_full.reshape(KO, 128, 2 * HPC * Dh).transpose(1, 0, 2)),
            "wv": np.ascontiguousarray(
                wv_full.reshape(KO, 128, HPC * Dh).transpose(1, 0, 2)),
            "wout": np.ascontiguousarray(
                Wout[csl, :].astype(ml_dtypes.bfloat16)
                .reshape(2, 128, D).transpose(1, 0, 2)),
        })
    return in_maps


def kernel(x, causal_mask, key_padding_mask, Wqkv, bqkv, Wout, bout,
           _trace=False):
    from concourse.bass_utils import run_bass_kernel_spmd

    x = np.asarray(x, dtype=np.float32)
    Wqkv = np.asarray(Wqkv, dtype=np.float32)
    Wout = np.asarray(Wout, dtype=np.float32)
    bqkv = np.asarray(bqkv, dtype=np.float32)
    bout = np.asarray(bout, dtype=np.float32)
    if np.any(np.asarray(key_padding_mask)):
        raise NotImplementedError("key_padding_mask with padded keys")
    if np.any(bqkv):
        raise NotImplementedError("nonzero bqkv")

    nc = _get_program()
    in_maps = make_in_maps(x, Wqkv, Wout)
    res = run_bass_kernel_spmd(nc, in_maps, core_ids=list(range(NCORES)),
                               trace=_trace)
    G = NCORES // B
    outp = np.empty((B, T, D), dtype=np.float32)
    for b in range(B):
        acc = res.results[b * G]["out"].astype(np.float32)
        for hg in range(1, G):
            acc = acc + res.results[b * G + hg]["out"].astype(np.float32)
        outp[b] = acc + bout
    kernel.last_exec_time_ns = res.exec_time_ns
    return outp
